# revision 45
# baseline (speedup 1.0000x reference)
"""GatedCrossAttentionBlock Trainium2 kernel, SPMD over 8 NeuronCores.

Sharding: core c handles batch b=c//2, T1-half h=c%2 (1024 rows of T1).
No collectives. Activations feature-major (transposed); all big matmuls
fp8e4 DoubleRow (2x tensor throughput), accumulating f32 in PSUM.

Scale folding: the whole post-attention residual stream is carried
S2-scaled (S2 a power of two) so Wout/W2 quantization scales cost no
extra ops; host divides the output by S2. LayerNorm mean-subtraction is
folded into the projection matmuls as a rank-1 update (colsum(W) x
mu*rstd), so normalize is a single vector multiply per tile.
"""
import sys

for _p in ("/opt/trn_rl_repo", "/root/.axon_site/_ro/trn_rl_repo"):
    if _p not in sys.path:
        sys.path.insert(0, _p)

import numpy as np
import ml_dtypes
from contextlib import ExitStack

import concourse.bass as bass
from concourse import bacc
import concourse.mybir as mybir
import concourse.tile as tile

F32 = mybir.dt.float32
BF16 = mybir.dt.bfloat16
FP8 = mybir.dt.float8e4
AF = mybir.ActivationFunctionType
ALU = mybir.AluOpType
DR = mybir.MatmulPerfMode.DoubleRow

B, T1, TKV, N_, DIM, DL, DH, H, MULT = 4, 2048, 8, 64, 1024, 1024, 64, 8, 4
J = TKV * N_          # 512
INNER = H * DH        # 512
DFF = MULT * DIM      # 4096
TI = 1024             # T1 rows per core
NBLK = 2              # i-blocks of 512 per core
CT = DIM // 128       # 8 c-tiles
TINY = 1e-30
EPS = 1e-5

_nc_cache = None
_nc_key = None


def build_nc(SQ, SKV, S1, S2):
    nc = bacc.Bacc()
    d_qoT = nc.declare_dram_parameter("qoT", [DIM, TI], BF16, isOutput=False)
    d_kvq = nc.declare_dram_parameter("kvq", [128, 8 * J], FP8, isOutput=False)
    d_mask = nc.declare_dram_parameter("mask01", [128, 4 * TI], FP8,
                                       isOutput=False)
    d_qm = nc.declare_dram_parameter("qmaskT", [1, TI], F32, isOutput=False)
    d_wgq = nc.declare_dram_parameter("wgq", [128, 8 * INNER], FP8,
                                      isOutput=False)
    d_cwg = nc.declare_dram_parameter("cwg", [1, 2 * INNER], FP8,
                                      isOutput=False)
    d_wqv = nc.declare_dram_parameter("wqv", [128, 4], F32, isOutput=False)
    d_wkvq = nc.declare_dram_parameter("wkvq", [128, 8 * 2 * INNER], FP8,
                                       isOutput=False)
    d_woq = nc.declare_dram_parameter("woq", [128, 4 * DIM], FP8,
                                      isOutput=False)
    d_w1q = nc.declare_dram_parameter("w1q", [128, 8 * DFF], FP8,
                                      isOutput=False)
    d_id8 = nc.declare_dram_parameter("id8", [128, 512], FP8, isOutput=False)
    d_w1v = nc.declare_dram_parameter("w1v", [128, 32], F32, isOutput=False)
    d_w2q = nc.declare_dram_parameter("w2q", [128, 8 * 32 * 128], FP8,
                                      isOutput=False)
    d_out = nc.declare_dram_parameter("out", [DIM, TI], F32, isOutput=True)

    with tile.TileContext(nc) as tc, ExitStack() as ctx:
        pers = ctx.enter_context(tc.tile_pool(name="pers", bufs=1))
        # ---------------- persistent tiles ----------------
        xT = [pers.tile([128, TI], F32, tag=f"xT{t}", name=f"xT{t}")
              for t in range(CT)]
        # LN output, fp8, DoubleRow layout: tile tp holds chunks 2tp, 2tp+1.
        xc8 = [pers.tile([128, 2, TI], FP8, tag=f"xc{t}", name=f"xc{t}")
               for t in range(4)]
        w1q_sb = pers.tile([128, 8, DFF], FP8, tag="w1q", name="w1q_sb")
        cwg_sb = pers.tile([1, 2, INNER], FP8, tag="cwg", name="cwg_sb")
        id8_sb = pers.tile([128, 2, 256], FP8, tag="id8", name="id8_sb")
        wqv_sb = pers.tile([128, 4], F32, tag="wqv", name="wqv_sb")
        w1v_sb = pers.tile([128, 32], F32, tag="w1v", name="w1v_sb")
        qm_sb = pers.tile([1, TI], F32, tag="qm", name="qm_sb")
        ones_c = pers.tile([128, 1], BF16, tag="ones_c", name="ones_c")
        ones_r = pers.tile([1, 128], BF16, tag="ones_r", name="ones_r")
        eps_t = pers.tile([1, 1], F32, tag="eps_t", name="eps_t")
        # rank-1 LN mean-correction operand: fp8 pairs [(−mu·rstd·64), 0]
        # so the correction matmul rides the DoubleRow path with cw/64.
        nmr1 = pers.tile([1, 2, TI], FP8, tag="nmr1", name="nmr1")
        nc.vector.memset(nmr1[:, 1, :], 0.0)
        nc.vector.memset(ones_c[:], 1.0)
        nc.vector.memset(ones_r[:], 1.0)
        nc.vector.memset(eps_t[:], EPS * S2 * S2)

        scr = ctx.enter_context(tc.tile_pool(name="scr", bufs=3))

        def ln_stats_tile(st, t, src_tile, src_bf):
            if src_bf:
                cbf = src_tile
            else:
                cbf = scr.tile([128, TI], BF16, tag="statbf",
                               name="statbf", bufs=2)
                nc.vector.tensor_copy(cbf[:], src_tile[:])
            sq = scr.tile([128, TI], BF16, tag="statsq", name="statsq",
                          bufs=2)
            nc.scalar.square(sq[:], cbf[:])
            for b in range(NBLK):
                sl = slice(b * 512, b * 512 + 512)
                nc.tensor.matmul(st[b][0:1, :], ones_c[:], cbf[:, sl],
                                 start=(t == 0), stop=(t == CT - 1))
                nc.tensor.matmul(st[b][32:33, :], ones_c[:],
                                 sq[:, sl], start=(t == 0),
                                 stop=(t == CT - 1))

        def ln_finish(pa, ps_stat, st, rb_sb, tag, nmr=None, nm_sb=None):
            """[1,TI] stats chain. Writes rb_sb [128,TI] f32 (rstd bcast)
            and either nmr (fp8 DR rank-1 operand, -mu*rstd*64) or nm_sb
            ([128,TI] f32 broadcast of -mu*rstd)."""
            mu = pa.tile([1, TI], F32, tag="st_mu", name=f"mu{tag}")
            ex2 = pa.tile([1, TI], F32, tag="st_ex2", name=f"ex2{tag}")
            for b in range(NBLK):
                sl = slice(b * 512, b * 512 + 512)
                nc.vector.tensor_scalar_mul(mu[:, sl], st[b][0:1, :],
                                            1.0 / DIM)
                nc.vector.tensor_scalar_mul(ex2[:, sl], st[b][32:33, :],
                                            1.0 / DIM)
            musq = pa.tile([1, TI], F32, tag="st_musq", name=f"musq{tag}")
            nc.vector.tensor_mul(musq[:], mu[:], mu[:])
            var = pa.tile([1, TI], F32, tag="st_var", name=f"var{tag}")
            nc.vector.tensor_sub(var[:], ex2[:], musq[:])
            std = pa.tile([1, TI], F32, tag="st_musq", name=f"std{tag}")
            nc.scalar.activation(std[:], var[:], AF.Sqrt, bias=eps_t[:])
            r = pa.tile([1, TI], F32, tag="st_ex2", name=f"r{tag}")
            nc.vector.reciprocal_approx_fast(r[:], std[:])
            r_bf = pa.tile([1, TI], BF16, tag="st_rbf", name=f"rbf{tag}")
            nc.vector.tensor_copy(r_bf[:], r[:])
            nmrf = pa.tile([1, TI], F32, tag="st_var", name=f"nmrf{tag}")
            nc.vector.tensor_mul(nmrf[:], mu[:], r[:])
            if nmr is not None:
                # nmr pair 0 = -mu * rstd * 64 (cw is pre-divided by 64)
                nc.vector.tensor_scalar_mul(nmr[:, 0, :], nmrf[:], -64.0)
            nm_bf = None
            if nm_sb is not None:
                nm_bf = pa.tile([1, TI], BF16, tag="st_nmbf",
                                name=f"nmbf{tag}")
                nc.vector.tensor_scalar_mul(nm_bf[:], nmrf[:], -1.0)
            for b in range(NBLK):
                sl = slice(b * 512, b * 512 + 512)
                rb_ps = ps_stat.tile([128, 512], F32, tag="rbb",
                                     name=f"rbps{tag}{b}", bufs=2)
                nc.tensor.matmul(rb_ps[:], ones_r[:], r_bf[:, sl],
                                 start=True, stop=True)
                nc.vector.tensor_copy(rb_sb[:, sl], rb_ps[:])
                if nm_sb is not None:
                    nm_ps = ps_stat.tile([128, 512], F32, tag="rbb",
                                         name=f"nmps{tag}{b}", bufs=2)
                    nc.tensor.matmul(nm_ps[:], ones_r[:], nm_bf[:, sl],
                                     start=True, stop=True)
                    nc.vector.tensor_copy(nm_sb[:, sl], nm_ps[:])

        def norm_mul(src_tiles, rb_sb):
            for t in range(CT):
                nc.vector.tensor_mul(xc8[t // 2][:, t % 2, :],
                                     src_tiles[t][:], rb_sb[:])

        with tc.tile_pool(name="attn", bufs=1) as pa:
            qoT = [pa.tile([128, TI], BF16, tag=f"qoT{t}", name=f"qoT{t}")
                   for t in range(CT)]
            kv_sb = pa.tile([128, 8, J], FP8, tag="kv", name="kv_sb")
            mask_sb = pa.tile([128, 4, TI], FP8, tag="mask", name="mask_sb")
            wgq_sb = pa.tile([128, 8, INNER], FP8, tag="wgq", name="wgq_sb")
            wkvq_sb = pa.tile([128, 8, 2 * INNER], FP8, tag="wkvq",
                              name="wkvq_sb")
            woq_sb = pa.tile([128, 4, DIM], FP8, tag="woq", name="woq_sb")
            rb1_sb = pa.tile([128, TI], F32, tag="rb1", name="rb1_sb")
            rb2_sb = pa.tile([128, TI], F32, tag="rb1", name="rb2_sb")
            # plane layout: tile g, partition 32m+p, pair-index i holds
            # head 4g+m, dh=32i+p — so sim contracts dh as 32 partitions x 2
            # DoubleRow subtiles (weights are column-permuted host-side).
            qT8 = [pa.tile([128, 2, TI], FP8, tag=f"qT{g}", name=f"qT{g}")
                   for g in range(2)]
            kT8 = [pa.tile([128, 2, J], FP8, tag=f"kT{g}", name=f"kT{g}")
                   for g in range(2)]
            # per-head stride padded to 72 so DoubleRow ldweights APs stay
            # even-sized and even-aligned (65 is rejected by codegen)
            VP = 72
            v_aug = [pa.tile([128, 2, H, VP], FP8, tag=f"vaug{j}",
                             name=f"vaug{j}") for j in range(2)]
            attn_cat = [pa.tile([128, 2, TI], FP8, tag=f"acat{d}",
                                name=f"acat{d}") for d in range(2)]

            for t in range(CT):
                nc.sync.dma_start(out=qoT[t],
                                  in_=d_qoT[t * 128:(t + 1) * 128, :])
            nc.sync.dma_start(out=kv_sb,
                              in_=d_kvq.rearrange("p (a j) -> p a j", a=8))
            nc.sync.dma_start(out=mask_sb,
                              in_=d_mask.rearrange("p (a t) -> p a t", a=4))
            nc.sync.dma_start(out=qm_sb, in_=d_qm[:, :])
            nc.sync.dma_start(out=wgq_sb,
                              in_=d_wgq.rearrange("p (a n) -> p a n", a=8))
            nc.sync.dma_start(out=wkvq_sb,
                              in_=d_wkvq.rearrange("p (a n) -> p a n", a=8))
            nc.sync.dma_start(out=cwg_sb,
                              in_=d_cwg.rearrange("p (a n) -> p a n", a=2))
            nc.sync.dma_start(out=wqv_sb, in_=d_wqv[:, :])
            nc.sync.dma_start(out=id8_sb,
                              in_=d_id8.rearrange("p (a n) -> p a n", a=2))
            nc.sync.dma_start(out=w1v_sb, in_=d_w1v[:, :])
            nc.sync.dma_start(out=woq_sb,
                              in_=d_woq.rearrange("p (a n) -> p a n", a=4))
            nc.sync.dma_start(out=w1q_sb,
                              in_=d_w1q.rearrange("p (a n) -> p a n", a=8))

            for jp in range(2):
                nc.vector.memset(v_aug[jp][:, :, :, DH:DH + 1], 1.0)
                nc.vector.memset(v_aug[jp][:, :, :, DH + 1:VP], 0.0)

            # ---- LN1 stats + k/v projections ----
            with tc.tile_pool(name="psStat", bufs=1, space="PSUM") as psStat, \
                 tc.tile_pool(name="psKV", bufs=2, space="PSUM") as psKV:
                st1 = [psStat.tile([33, 512], F32, tag=f"stat{b}",
                                   name=f"st1{b}") for b in range(NBLK)]
                for t in range(CT):
                    ln_stats_tile(st1, t, qoT[t], True)
                for d in range(4):
                    k_ps = psKV.tile([128, 512], F32, tag="kv", name="k_ps")
                    for tp in range(4):
                        nc.tensor.matmul(
                            k_ps[:],
                            wkvq_sb[:, 2 * tp:2 * tp + 2,
                                    d * 128:(d + 1) * 128],
                            kv_sb[:, 2 * tp:2 * tp + 2, :],
                            start=(tp == 0), stop=(tp == 3), perf_mode=DR)
                    nc.scalar.activation(kT8[d // 2][:, d % 2, :], k_ps[:],
                                         AF.Copy, scale=1.0 / SKV)
                for c in range(4):
                    v_ps = psKV.tile([128, 512], F32, tag="kv", name="v_ps")
                    for tp in range(4):
                        nc.tensor.matmul(
                            v_ps[:],
                            kv_sb[:, 2 * tp:2 * tp + 2,
                                  c * 128:(c + 1) * 128],
                            wkvq_sb[:, 2 * tp:2 * tp + 2, INNER:2 * INNER],
                            start=(tp == 0), stop=(tp == 3), perf_mode=DR)
                    nc.vector.tensor_scalar_mul(
                        v_aug[c // 2][:, c % 2, :, 0:DH],
                        v_ps[:].rearrange("p (h d) -> p h d", h=H),
                        1.0 / SKV)
                ln_finish(pa, psStat, st1, rb1_sb, "1", nmr=nmr1)
            norm_mul(qoT, rb1_sb)

            # ---- q projection ----
            with tc.tile_pool(name="psQ", bufs=2, space="PSUM") as psQ:
                for d in range(4):
                    q_ps = psQ.tile([128, 2, 512], F32, tag="q", name="q_ps")
                    for b in range(NBLK):
                        sl = slice(b * 512, b * 512 + 512)
                        for tp in range(4):
                            nc.tensor.matmul(
                                q_ps[:, b, :],
                                wgq_sb[:, 2 * tp:2 * tp + 2,
                                       d * 128:(d + 1) * 128],
                                xc8[tp][:, :, sl],
                                start=(tp == 0), stop=False, perf_mode=DR)
                        nc.tensor.matmul(q_ps[:, b, :],
                                         cwg_sb[:, :, d * 128:(d + 1) * 128],
                                         nmr1[:, :, sl],
                                         start=False, stop=True, perf_mode=DR)
                    for b in range(NBLK):
                        sl = slice(b * 512, b * 512 + 512)
                        nc.vector.tensor_scalar(qT8[d // 2][:, d % 2, sl],
                                                q_ps[:, b, :],
                                                1.0 / SQ, wqv_sb[:, d:d + 1],
                                                op0=ALU.mult, op1=ALU.add)

            # ---- attention ----
            with tc.tile_pool(name="psS", bufs=2, space="PSUM") as psS, \
                 tc.tile_pool(name="psAv", bufs=2, space="PSUM") as psAv:
                for h in range(H):
                    g, m = h // 4, h % 4
                    pr = slice(32 * m, 32 * m + 32)
                    row = 64 * (h % 2)
                    for b in range(NBLK):
                        sl = slice(b * 512, b * 512 + 512)
                        av_ps = psAv.tile([VP, 512], F32, tag="av",
                                          name="av_ps")
                        for jp in range(2):
                            s_ps = psS.tile([128, 2, 512], F32, tag="sim",
                                            name="s_ps")
                            for i in range(2):
                                jc = 2 * jp + i
                                nc.tensor.matmul(
                                    s_ps[:, i, :],
                                    kT8[g][pr, :, jc * 128:(jc + 1) * 128],
                                    qT8[g][pr, :, sl],
                                    start=True, stop=False, perf_mode=DR,
                                    tile_position=(32 * m, 0))
                                # accumulate the additive mask (0 / -240)
                                # through an fp8 identity DoubleRow matmul
                                nc.tensor.matmul(
                                    s_ps[:, i, :],
                                    id8_sb[:, :, 128 * i:128 * i + 128],
                                    mask_sb[:, 2 * jp:2 * jp + 2, sl],
                                    start=False, stop=True, perf_mode=DR)
                            pq = scr.tile([128, 2, 512], FP8, tag="pq",
                                          name="pq", bufs=3)
                            nc.scalar.activation(pq[:], s_ps[:], AF.Exp)
                            nc.tensor.matmul(av_ps[:],
                                             v_aug[jp][:, :, h, :], pq[:],
                                             start=(jp == 0), stop=(jp == 1),
                                             perf_mode=DR)
                        s_t = scr.tile([1, 512], F32, tag="s_t", name="s_t",
                                       bufs=2)
                        nc.vector.tensor_scalar_add(s_t[:],
                                                    av_ps[DH:DH + 1, :], TINY)
                        rec = scr.tile([1, 512], F32, tag="rec", name="rec",
                                       bufs=2)
                        nc.vector.reciprocal_approx_fast(rec[:], s_t[:])
                        r_bf = scr.tile([1, 512], BF16, tag="rbf_h",
                                        name="rbf_h", bufs=2)
                        nc.vector.tensor_mul(r_bf[:], rec[:], qm_sb[:, sl])
                        rb_ps = psAv.tile([64, 512], F32, tag="rb",
                                          name="rb_ps")
                        nc.tensor.matmul(rb_ps[:], ones_r[:, 0:64], r_bf[:],
                                         start=True, stop=True)
                        rb2 = scr.tile([64, 512], F32, tag="rb2", name="rb2",
                                       bufs=2)
                        nc.vector.tensor_copy(rb2[:], rb_ps[:])
                        nc.vector.tensor_mul(
                            attn_cat[h // 4][row:row + 64, (h // 2) % 2, sl],
                            av_ps[0:DH, :], rb2[:])

            # ---- Wout + residual, with LN2 stats interleaved per tile ----
            nm2_sb = pa.tile([128, TI], F32, tag="nm2", name="nm2_sb")
            with tc.tile_pool(name="psC", bufs=2, space="PSUM") as psC, \
                 tc.tile_pool(name="psStat2", bufs=1, space="PSUM") as psS2:
                st2 = [psS2.tile([33, 512], F32, tag=f"stat{b}",
                                 name=f"st2{b}") for b in range(NBLK)]
                for e in range(CT):
                    wo_ps = psC.tile([128, 2, 512], F32, tag="wo",
                                     name="wo_ps")
                    for b in range(NBLK):
                        sl = slice(b * 512, b * 512 + 512)
                        for dp in range(2):
                            nc.tensor.matmul(
                                wo_ps[:, b, :],
                                woq_sb[:, 2 * dp:2 * dp + 2,
                                       e * 128:(e + 1) * 128],
                                attn_cat[dp][:, :, sl],
                                start=(dp == 0), stop=(dp == 1), perf_mode=DR)
                    nc.vector.tensor_add(
                        xT[e][:], wo_ps[:].rearrange("p a t -> p (a t)"),
                        qoT[e][:])
                    ln_stats_tile(st2, e, xT[e], False)
                ln_finish(pa, psS2, st2, rb2_sb, "2", nm_sb=nm2_sb)
            # LN2 normalize: xc = x*rstd + (-mu*rstd), explicit on vector
            for t in range(CT):
                tmp = scr.tile([128, TI], BF16, tag="nmtmp", name="nmtmp",
                               bufs=2)
                nc.vector.tensor_mul(tmp[:], xT[t][:], rb2_sb[:])
                nc.vector.tensor_add(xc8[t // 2][:, t % 2, :], tmp[:],
                                     nm2_sb[:])

        # ---------------- FFN ----------------
        with tc.tile_pool(name="ffn", bufs=1) as pf, \
             tc.tile_pool(name="wstream", bufs=2) as ws, \
             tc.tile_pool(name="ostage", bufs=2) as ost, \
             tc.tile_pool(name="psH", bufs=3, space="PSUM") as psH:
            gT8 = [pf.tile([128, 2, TI], FP8, tag=f"gT{f}", name=f"gT{f}")
                   for f in range(16)]
            for f in range(32):
                h_ps = psH.tile([128, 2, 512], F32, tag="h", name="h_ps")
                for b in range(NBLK):
                    sl = slice(b * 512, b * 512 + 512)
                    for tp in range(4):
                        nc.tensor.matmul(
                            h_ps[:, b, :],
                            w1q_sb[:, 2 * tp:2 * tp + 2,
                                   f * 128:(f + 1) * 128],
                            xc8[tp][:, :, sl],
                            start=(tp == 0), stop=(tp == 3), perf_mode=DR)
                nc.scalar.activation(
                    gT8[f // 2][:, f % 2, :],
                    h_ps[:].rearrange("p a t -> p (a t)"),
                    AF.Gelu, bias=w1v_sb[:, f:f + 1], scale=1.0 / S1)
            w2r = d_w2q.rearrange("p (e a n) -> p e a n", e=8, a=32)
            for e in range(CT):
                w2t = ws.tile([128, 32, 128], FP8, tag="w2s", name="w2t")
                nc.sync.dma_start(out=w2t, in_=w2r[:, e])
                h2_ps = psH.tile([128, 2, 512], F32, tag="h", name="h2_ps")
                for b in range(NBLK):
                    sl = slice(b * 512, b * 512 + 512)
                    for fp in range(16):
                        nc.tensor.matmul(
                            h2_ps[:, b, :],
                            w2t[:, 2 * fp:2 * fp + 2, :],
                            gT8[fp][:, :, sl],
                            start=(fp == 0), stop=(fp == 15), perf_mode=DR)
                stg = ost.tile([128, TI], F32, tag="stg", name="stg")
                nc.vector.tensor_add(stg[:],
                                     h2_ps[:].rearrange("p a t -> p (a t)"),
                                     xT[e][:])
                nc.sync.dma_start(out=d_out[e * 128:(e + 1) * 128, :],
                                  in_=stg[:])
    nc.compile()
    return nc


def _pow2floor(x):
    return float(2.0 ** np.floor(np.log2(x)))


def _q8(x, s):
    return np.clip(np.asarray(x, np.float64) * s, -240.0, 240.0).astype(
        ml_dtypes.float8_e4m3)


def _pack_rows(w8):
    """[(a*128+p), n] -> [p, (a n)] packed fp8 array."""
    a = w8.shape[0] // 128
    return np.ascontiguousarray(
        w8.reshape(a, 128, -1).transpose(1, 0, 2).reshape(128, -1))


def _scales(inputs):
    f64 = np.float64
    scale = DH ** (-0.5)
    tanh_a = np.tanh(f64(inputs["attn_gate"][0]))
    tanh_f = np.tanh(f64(inputs["ff_gate"][0]))
    Wg = inputs["ln_g"].astype(f64)[:, None] * inputs["Wq"].astype(f64) * scale
    W1g = inputs["ff_ln_g"].astype(f64)[:, None] * inputs["W1"].astype(f64)
    SQ = _pow2floor(224.0 / np.abs(Wg).max())
    SKV = _pow2floor(224.0 / np.abs(inputs["Wkv"]).max())
    S1 = _pow2floor(224.0 / np.abs(W1g).max())
    S2 = min(_pow2floor(224.0 / np.abs(inputs["Wout"] * tanh_a).max()),
             _pow2floor(224.0 / np.abs(inputs["W2"] * tanh_f).max()))
    return SQ, SKV, S1, S2, Wg, W1g, tanh_a, tanh_f


def _prep_in_maps(inputs, SQ, SKV, S1, S2, Wg, W1g, tanh_a, tanh_f):
    bf = ml_dtypes.bfloat16
    f64 = np.float64
    scale = DH ** (-0.5)
    qo = inputs["qo"]
    kvo = inputs["kvo"]
    attn_mask = inputs["attn_mask"]
    q_mask = inputs["q_mask"]
    kv_mask = inputs["kv_mask"]

    # plane permutation: old col n = h*64+dh -> new col (2g+i)*128+32m+p
    # with h=4g+m, dh=32i+p (sim contracts dh via 32 partitions x 2 DR)
    n = np.arange(INNER)
    h_, dh_ = n // 64, n % 64
    newidx = (2 * (h_ // 4) + dh_ // 32) * 128 + 32 * (h_ % 4) + dh_ % 32
    Wg_p = np.empty_like(Wg)
    Wg_p[:, newidx] = Wg
    wgq = _q8(Wg_p, SQ)
    cwg = wgq.astype(np.float32).sum(axis=0)
    cw8g = np.zeros((1, 2 * INNER), dtype=ml_dtypes.float8_e4m3)
    cw8g[0, :INNER] = _q8(cwg / 64.0, 1.0)
    wqv = (inputs["ln_b"].astype(f64) @ inputs["Wq"].astype(f64) * scale)
    wqv_p = np.empty_like(wqv)
    wqv_p[newidx] = wqv
    Wkv_p = np.array(inputs["Wkv"], dtype=f64)
    Wkv_p[:, newidx] = Wkv_p[:, :INNER].copy()
    wkvq = _q8(Wkv_p, SKV)
    woq = _q8(inputs["Wout"].astype(f64) * tanh_a * S2, 1.0)
    w1q = _q8(W1g, S1)
    w1v = (inputs["ff_ln_b"].astype(f64) @ inputs["W1"].astype(f64))
    # fp8 DR identity pair tile: (I,0) at cols 0:128, (0,I) at cols 128:256
    id8 = np.zeros((128, 2, 256), dtype=np.float64)
    id8[np.arange(128), 0, np.arange(128)] = 1.0
    id8[np.arange(128), 1, 128 + np.arange(128)] = 1.0
    id8 = id8.reshape(128, 512).astype(ml_dtypes.float8_e4m3)
    w2q = _q8(inputs["W2"].astype(f64) * tanh_f * S2, 1.0)
    # w2 packed [p, (e a n)]
    w2p = np.ascontiguousarray(
        w2q.reshape(32, 128, 8, 128).transpose(1, 2, 0, 3).reshape(128, -1))
    shared = {
        "wgq": _pack_rows(wgq),
        "cwg": cw8g,
        "wqv": np.ascontiguousarray(wqv_p.reshape(4, 128).T,
                                    dtype=np.float32),
        "wkvq": _pack_rows(wkvq),
        "woq": _pack_rows(woq),
        "w1q": _pack_rows(w1q),
        "id8": id8,
        "w1v": np.ascontiguousarray(w1v.reshape(32, 128).T,
                                    dtype=np.float32),
        "w2q": w2p,
    }
    in_maps = []
    for c in range(8):
        b, hf = c // 2, c % 2
        rows = slice(hf * TI, (hf + 1) * TI)
        m = (attn_mask[b, rows, :] & kv_mask[b].reshape(J)[None, :])
        mask01 = _pack_rows(
            np.where(m.T, 0.0, -240.0).astype(ml_dtypes.float8_e4m3))
        kvoT = np.asarray(kvo[b], np.float32).reshape(J, DL).T
        im = dict(shared)
        im["qoT"] = np.ascontiguousarray(qo[b, rows, :].T * np.float32(S2),
                                         dtype=bf)
        im["kvq"] = _pack_rows(_q8(kvoT, 1.0))
        im["mask01"] = mask01
        im["qmaskT"] = np.ascontiguousarray(q_mask[b, rows][None, :],
                                            dtype=np.float32)
        in_maps.append(im)
    return in_maps


def kernel(**inputs):
    global _nc_cache, _nc_key
    inputs = {k: np.asarray(v) for k, v in inputs.items()}
    SQ, SKV, S1, S2, Wg, W1g, tanh_a, tanh_f = _scales(inputs)
    in_maps = _prep_in_maps(inputs, SQ, SKV, S1, S2, Wg, W1g, tanh_a, tanh_f)
    key = (SQ, SKV, S1, S2)
    if _nc_cache is None or _nc_key != key:
        _nc_cache = build_nc(SQ, SKV, S1, S2)
        _nc_key = key
    from concourse.bass_utils import run_bass_kernel_spmd
    res = run_bass_kernel_spmd(_nc_cache, in_maps, list(range(8)))
    out = np.empty((B, T1, DIM), dtype=np.float32)
    inv = np.float32(1.0 / S2)
    for c in range(8):
        b, hf = c // 2, c % 2
        out[b, hf * TI:(hf + 1) * TI, :] = res.results[c]["out"].T * inv
    return out


if __name__ == "__main__":
    nc = build_nc(2.0 ** 14, 2.0 ** 11, 2.0 ** 13, 2.0 ** 14)
    print("built ok")
